# revision 3
# baseline (speedup 1.0000x reference)
"""Stereo cost-volume builder (nn_CostBuilder) as a Trainium2 Bass kernel.

Reference op: out[b, 0:C,  d, h, w] = left[b, c, h, w]   * (w >= d)
              out[b, C:2C, d, h, w] = right[b, c, h, w-d] * (w >= d)
with B=4, C=32, D=48, H=64, W=128 (f32). Output is [4, 64, 48, 64, 128].

Sharding across 8 cores: core m -> (b = m//2, d-half = m%2). Each core
produces out[b, :, d0:d0+24, :, :] (50.3 MB), i.e. both the left-masked and
right-shifted channels for 24 of the 48 disparities. This keeps the program
uniform (true SPMD) because the disparity offset d0 only changes per-core
*data*: the mask tensor and the host-side shift applied to the padded right
features.

Per-core device program (memory-regime: ~50 MB of output writes dominate):
  - left:  DVE tensor_mul of the input row (broadcast along d, stride-0 AP)
           with a precomputed {0,1} mask, staged in SBUF, then DMA out.
  - right: no compute at all - a single DMA per tile reads the zero-padded
           right rows with a (d: step -1, w: step +1) gather AP, which
           realizes the shift-by-d with zero fill directly from SBUF.
"""

import sys

if "/opt/trn_rl_repo" not in sys.path:
    sys.path.insert(0, "/opt/trn_rl_repo")

import numpy as np

import concourse.bass as bass
import concourse.bacc as bacc
import concourse.mybir as mybir
import concourse.tile as tile
from concourse.bass_utils import run_bass_kernel_spmd

B, C, H, W = 4, 32, 64, 128
D = 48          # MAX_DISP // 4
DD = D // 2     # disparities per core
N_CORES = 8
PAD = DD + DD + W  # 176: zero prefix covers max host shift (24+d0<=48) + reads down to col 1

_NC_CACHE = {}


def _build_nc():
    nc = bacc.Bacc("TRN2", target_bir_lowering=False, debug=False)
    f32 = mybir.dt.float32

    lfeat = nc.dram_tensor("lfeat", [C, H, W], f32, kind="ExternalInput").ap()
    rpad = nc.dram_tensor("rpad", [C, H, PAD], f32, kind="ExternalInput").ap()
    lmask = nc.dram_tensor("lmask", [128, DD * W], f32, kind="ExternalInput").ap()
    out = nc.dram_tensor("out", [2 * C, DD, H, W], f32, kind="ExternalOutput").ap()

    c_str = DD * H * W  # element stride of the channel dim in `out`

    with tile.TileContext(nc) as tc:
        with (
            tc.tile_pool(name="mask", bufs=1) as mask_pool,
            tc.tile_pool(name="ins", bufs=3) as in_pool,
            tc.tile_pool(name="stage", bufs=3) as stage_pool,
        ):
            mtile = mask_pool.tile([128, DD * W], f32, name="mtile")
            nc.sync.dma_start(mtile[:], lmask)

            for t in range(C // 2):
                c0 = 2 * t
                # 128 partitions = (2 channels) x (64 h rows)
                ltile = in_pool.tile([128, W], f32, name="ltile")
                nc.gpsimd.dma_start(ltile[:], lfeat[c0 : c0 + 2])
                rtile = in_pool.tile([128, PAD], f32, name="rtile")
                nc.gpsimd.dma_start(rtile[:], rpad[c0 : c0 + 2])

                # left: stage[p, d, w] = lfeat[p, w] * mask[d, w]
                lstage = stage_pool.tile([128, DD * W], f32, name="lstage")
                nc.vector.tensor_mul(
                    lstage[:].rearrange("p (d w) -> p d w", d=DD),
                    ltile[:].unsqueeze(1).to_broadcast((128, DD, W)),
                    mtile[:].rearrange("p (d w) -> p d w", d=DD),
                )
                # DMA APs are limited to 3 dims, so issue one DMA per channel
                # (64 partitions each): dst [H, DD, W], src matches.
                sL = lstage[:]
                sR = rtile[:, DD : DD + 1]
                for cc in range(2):
                    srcL = bass.AP(
                        sL.tensor,
                        sL.offset + cc * H * DD * W,
                        [[DD * W, H], [W, DD], [1, W]],
                    )
                    dstL = out[c0 + cc].transpose([1, 0, 2])  # [H, DD, W]
                    nc.sync.dma_start(dstL, srcL)

                    # right: gather DMA straight out of the padded input rows.
                    # src col = DD + w - d; zero prefix of rpad fills w < d.
                    srcR = bass.AP(
                        sR.tensor,
                        sR.offset + cc * H * PAD,
                        [[PAD, H], [-1, DD], [1, W]],
                    )
                    dstR = out[C + c0 + cc].transpose([1, 0, 2])
                    nc.scalar.dma_start(dstR, srcR)

    nc.compile()
    return nc


def get_nc():
    if "nc" not in _NC_CACHE:
        _NC_CACHE["nc"] = _build_nc()
    return _NC_CACHE["nc"]


def make_in_maps(left, right):
    """Per-core input dicts for run_bass_kernel_spmd."""
    left = np.ascontiguousarray(left, dtype=np.float32)
    right = np.ascontiguousarray(right, dtype=np.float32)
    ds = np.arange(DD)[:, None]
    w = np.arange(W)[None, :]
    in_maps = []
    for m in range(N_CORES):
        b, dh = divmod(m, 2)
        d0 = DD * dh
        rpad = np.zeros((C, H, PAD), np.float32)
        rpad[:, :, DD + d0 : DD + d0 + W] = right[b]
        mrow = (w >= (d0 + ds)).astype(np.float32).reshape(1, DD * W)
        lmask = np.ascontiguousarray(np.broadcast_to(mrow, (128, DD * W)))
        in_maps.append(
            {"lfeat": np.ascontiguousarray(left[b]), "rpad": rpad, "lmask": lmask}
        )
    return in_maps


def assemble(results):
    """Gather per-core [2C, DD, H, W] chunks into the full [B, 2C, D, H, W]."""
    full = np.empty((B, 2 * C, D, H, W), np.float32)
    for m in range(N_CORES):
        b, dh = divmod(m, 2)
        full[b, :, DD * dh : DD * dh + DD] = results[m]["out"]
    return full


def kernel(**inputs):
    nc = get_nc()
    in_maps = make_in_maps(inputs["left_feats"], inputs["right_feats"])
    res = run_bass_kernel_spmd(nc, in_maps, list(range(N_CORES))).results
    return assemble(res)
